# revision 20
# baseline (speedup 1.0000x reference)
"""GQA attention layer (B=2, S=2048, D=4096, 32 Q heads / 8 KV heads, RoPE,
causal) on 8 Trainium2 NeuronCores, tensor-parallel over heads.

Each core owns 4 Q heads + 1 KV head: it computes its Q/K/V projections,
RoPE, causal attention, and a partial output projection (rank-512 slice of
the wo contraction).  The host sums the 8 partial outputs.

v2 (all-bf16, SBUF-resident):
- every SBUF operand in bf16 (fp32 PSUM accumulation): halves DMA traffic,
  enables fast weight loads, and lowers PE power draw (the fp32r baseline
  ran power-throttled at K=13/16 for half the kernel).
- q/k/v and the token-major V live entirely in SBUF; no DRAM round trip
  between the projection and attention phases.
- causal trimming: diagonal score/AV/sum matmuls shrink their moving
  width to 512/384/256/128 instead of running fully masked.
- softmax normalization folded into the PSUM eviction of the attention
  output (broadcast + reciprocal + one fused multiply).
- PE warm-up matmuls gated only on a tiny mask DMA so the HAM clock gate
  opens before the real work arrives; weights stream per k-chunk so the
  first projection matmul does not wait for whole-tensor DMAs.
"""

import os
import sys
import types
from contextlib import ExitStack

import numpy as np
import ml_dtypes

import concourse.bass as bass
import concourse.tile as tile
from concourse import bacc
from concourse import mybir
from concourse import bass_utils
from concourse.bass_utils import run_bass_kernel_spmd

# ---------------------------------------------------------------------------
# Optional NTFF profiling support under axon. The trimmed image's `antenv`
# lacks `axon_hooks`, so run_bass_kernel_spmd(trace=True) would silently skip
# tracing; register the hook ourselves. Harmless when unavailable.
try:
    import antenv  # noqa: F401
    from trn_agent_boot.trn_boot import _ntff_profile_via_ctypes

    if "antenv.axon_hooks" not in sys.modules:
        _hooks_mod = types.ModuleType("antenv.axon_hooks")
        _hook = _ntff_profile_via_ctypes("/opt/axon/libaxon_pjrt.so")
        _hooks_mod.get_axon_ntff_profile_hook = lambda: _hook
        _hooks_mod.set_axon_ntff_profile_hook = lambda h: None
        sys.modules["antenv.axon_hooks"] = _hooks_mod
    bass_utils.upload_artifacts = lambda tmpdir: "local://skipped"
except Exception:
    pass

F32 = mybir.dt.float32
BF16 = mybir.dt.bfloat16
EXP = mybir.ActivationFunctionType.Exp
NPBF16 = ml_dtypes.bfloat16

B, S, D = 2, 2048, 4096
NH, NKV, HD = 32, 8, 128
T = B * S                       # 4096 tokens total
N_CORES = 8
QH = NH // N_CORES              # 4 local q heads
FL = QH * HD                    # 512 local q features
SCALE = 1.0 / float(np.sqrt(HD))
NEG = -1.0e30

NW = 512                        # token-group width in the QKV projection
QB = 512                        # q-block width in attention
DKD = D // 128                  # 32 contraction chunks for projections


def _build_program():
    nc = bacc.Bacc("TRN2", target_bir_lowering=False, debug=False,
                   num_devices=N_CORES)

    xT = nc.dram_tensor("xT", [D, T], BF16, kind="ExternalInput").ap()
    # weights pre-packed on host, k-chunk-major: [128, n_chunks * width]
    wqP = nc.dram_tensor("wqP", [128, DKD * FL], BF16, kind="ExternalInput").ap()
    wkP = nc.dram_tensor("wkP", [128, DKD * HD], BF16, kind="ExternalInput").ap()
    wvP = nc.dram_tensor("wvP", [128, DKD * HD], BF16, kind="ExternalInput").ap()
    woP = nc.dram_tensor("woP", [128, QH * D], BF16, kind="ExternalInput").ap()
    # RoPE constants, pre-assembled for the rotate-half formulation on the
    # even/odd-split feature layout: ropc = [cos; cos], rops = [-sin; sin].
    ropc = nc.dram_tensor("ropc", [HD, S], F32, kind="ExternalInput").ap()
    rops = nc.dram_tensor("rops", [HD, S], F32, kind="ExternalInput").ap()
    onesin = nc.dram_tensor("onesin", [128, 1], BF16, kind="ExternalInput").ap()
    # triangular mask for a diagonal 128x128 block: m[r, c] = 0 if c >= r
    maskin = nc.dram_tensor("maskin", [128, 128], BF16, kind="ExternalInput").ap()
    y = nc.dram_tensor("y", [T, D], BF16, kind="ExternalOutput").ap()

    with tile.TileContext(nc) as tc, ExitStack() as ctx:
        const = ctx.enter_context(tc.tile_pool(name="const", bufs=1))
        mtri = const.tile([128, 128], BF16)
        nc.sync.dma_start(mtri[:], maskin)
        ones_t = const.tile([128, 1], BF16)
        nc.sync.dma_start(ones_t[:], onesin)

        # PE warm-up: ~10us of dummy matmuls gated only on the tiny mask DMA,
        # so the HAM clock gate reaches K=8/8 before the first real matmul.
        with tc.tile_pool(name="warmps", bufs=1, space="PSUM") as wps:
            wtile = wps.tile([128, 128], F32)
            for _ in range(96):
                nc.tensor.matmul(wtile[:], mtri[:], mtri[:],
                                 start=True, stop=True)

        # Resident weights (DMAs emitted inside the phase-1 loop, interleaved
        # with the x-chunk loads so the sync queue serves them in need order).
        wpool = ctx.enter_context(tc.tile_pool(name="wqkv", bufs=1))
        wq_sb = wpool.tile([128, DKD * FL], BF16, tag="wq")
        wk_sb = wpool.tile([128, DKD * HD], BF16, tag="wk")
        wv_sb = wpool.tile([128, DKD * HD], BF16, tag="wv")

        # rope tables on the scalar hwdge queue, off the x/weight path
        rcpool = ctx.enter_context(tc.tile_pool(name="ropec", bufs=1))
        cos_s = rcpool.tile([HD, S], F32)
        nc.scalar.dma_start(cos_s[:], ropc)
        sin_s = rcpool.tile([HD, S], F32)
        nc.scalar.dma_start(sin_s[:], rops)

        # Resident activations (bf16, feature-major), split per batch so the
        # batch-0 attention does not depend on batch-1 projection writes.
        res = ctx.enter_context(tc.tile_pool(name="resident", bufs=1))
        qtb = [[res.tile([128, S], BF16, tag=f"qtb{h}_{b}", name=f"qtb{h}_{b}")
                for b in range(B)] for h in range(QH)]
        ktb = [res.tile([128, S], BF16, tag=f"ktb{b}", name=f"ktb{b}")
               for b in range(B)]
        Vt = [res.tile([128, S], BF16, tag=f"Vt{b}", name=f"Vt{b}")
              for b in range(B)]                  # token-major V blocks

        # ------------------------------------------------------------------
        # Phase 1: QKV projections + RoPE -> resident SBUF tiles
        # ------------------------------------------------------------------
        # Phase 1 runs in 1024-token group pairs: x chunks load once per pair
        # as [128, 1024] transfers (2 KiB rows) and stay resident for both
        # groups. Rope math for a group is deferred one group so its
        # swap-DMA-dependent vector ops never delay the PSUM-freeing copies.
        rtmp = ctx.enter_context(tc.tile_pool(name="ropetmp", bufs=2))
        with tc.tile_pool(name="xin", bufs=1) as xpool, \
             tc.tile_pool(name="qkvps", bufs=1, space="PSUM") as qkvps:

            def rope_math(xsb, xsw, out_ap, pos0):
                """out = RoPE(xsb) on the even/odd-split feature layout:
                out = x * [c;c] + swap(x) * [-s;s]; xsw = swap(x) (already
                staged by DMA)."""
                c = cos_s[:, pos0:pos0 + NW]
                s = sin_s[:, pos0:pos0 + NW]
                t1 = rtmp.tile([128, NW], BF16, tag="t1")
                nc.vector.tensor_mul(t1[:], xsw[:], s)
                nc.vector.tensor_mul(out_ap, xsb[:], c)
                nc.vector.tensor_add(out_ap, out_ap, t1[:])

            deferred = None
            pair_xts = None
            for n in range(T // NW):
                b = n // (S // NW)
                pos0 = (n * NW) % S
                g0 = n * NW                      # global column
                pcol = (n % 2) * NW              # column within the pair tile
                qps = [qkvps.tile([128, NW], F32, tag=f"qps{m}", name=f"qps{m}")
                       for m in range(QH)]
                kps = qkvps.tile([128, NW], F32, tag="kps")
                vps = qkvps.tile([128, NW], F32, tag="vps")
                if n % 2 == 0:
                    pair_xts = []
                for k in range(DKD):
                    if n == 0:
                        # weight chunks land just ahead of their x chunk
                        if k % 8 == 0:
                            q8 = 8 * FL
                            nc.sync.dma_start(
                                wq_sb[:, (k // 8) * q8:(k // 8 + 1) * q8],
                                wqP[:, (k // 8) * q8:(k // 8 + 1) * q8])
                        if k == 0:
                            nc.sync.dma_start(wk_sb[:], wkP)
                            nc.sync.dma_start(wv_sb[:], wvP)
                    if n % 2 == 0:
                        xt = xpool.tile([128, 2 * NW], BF16, tag=f"xt{k}",
                                        name=f"xt{k}")
                        nc.sync.dma_start(
                            xt[:], xT[k * 128:(k + 1) * 128, g0:g0 + 2 * NW])
                        pair_xts.append(xt)
                    else:
                        xt = pair_xts[k]
                    st = (k == 0)
                    sp = (k == DKD - 1)
                    for m in range(QH):
                        nc.tensor.matmul(
                            qps[m][:],
                            wq_sb[:, k * FL + m * 128:k * FL + (m + 1) * 128],
                            xt[:, pcol:pcol + NW], start=st, stop=sp)
                    nc.tensor.matmul(
                        kps[:], wk_sb[:, k * HD:(k + 1) * HD],
                        xt[:, pcol:pcol + NW], start=st, stop=sp)
                    nc.tensor.matmul(
                        vps[:], wv_sb[:, k * HD:(k + 1) * HD],
                        xt[:, pcol:pcol + NW], start=st, stop=sp)
                # PSUM-freeing copies first (split across ACT and DVE) so the
                # next group's matmuls are only gated on these, then the swap
                # DMAs; the swap-dependent math runs one group later.
                ev = []
                for m in range(QH):
                    xsb = rtmp.tile([128, NW], BF16, tag=f"xsb{m}",
                                    name=f"xsb{m}")
                    if m % 2 == 1:
                        nc.vector.tensor_copy(xsb[:], qps[m][:])
                    else:
                        nc.scalar.copy(xsb[:], qps[m][:])
                    xsw = rtmp.tile([128, NW], BF16, tag=f"xsw{m}",
                                    name=f"xsw{m}")
                    # swaps ride the gpsimd software DGE: queue-full waits
                    # there cannot block the scalar/vector compute FIFOs
                    nc.gpsimd.dma_start(xsw[0:64, :], xsb[64:128, :])
                    nc.gpsimd.dma_start(xsw[64:128, :], xsb[0:64, :])
                    ev.append((xsb, xsw, qtb[m][b][:, pos0:pos0 + NW]))
                xk = rtmp.tile([128, NW], BF16, tag="xsbk")
                nc.vector.tensor_copy(xk[:], kps[:])
                xkw = rtmp.tile([128, NW], BF16, tag="xswk")
                nc.gpsimd.dma_start(xkw[0:64, :], xk[64:128, :])
                nc.gpsimd.dma_start(xkw[64:128, :], xk[0:64, :])
                ev.append((xk, xkw, ktb[b][:, pos0:pos0 + NW]))
                xv = rtmp.tile([128, NW], BF16, tag="xsbv")
                nc.scalar.copy(xv[:], vps[:])
                for j in range(NW // 128):
                    nc.sync.dma_start_transpose(
                        Vt[b][:, pos0 + j * 128:pos0 + (j + 1) * 128],
                        xv[:, j * 128:(j + 1) * 128])
                if deferred is not None:
                    for xsb, xsw, out_ap in deferred[0]:
                        rope_math(xsb, xsw, out_ap, deferred[1])
                deferred = (ev, pos0)
            # the last group's rope math (batch-1 tail) is emitted in the
            # middle of phase 2's first block, after its congested swap DMAs
            # have landed, so it cannot delay the first attention vector ops
            epilogue = deferred

        # ------------------------------------------------------------------
        # Phase 2: attention + output projection (wo deferred one block)
        # ------------------------------------------------------------------
        with tc.tile_pool(name="wo", bufs=1) as wopool, \
             tc.tile_pool(name="ptiles", bufs=3) as ptpool, \
             tc.tile_pool(name="attn", bufs=2) as atpool, \
             tc.tile_pool(name="smax", bufs=2) as smpool, \
             tc.tile_pool(name="ystage", bufs=3) as ypool, \
             tc.tile_pool(name="sps2", bufs=1, space="PSUM") as sp2sum, \
             tc.tile_pool(name="sps1", bufs=1, space="PSUM") as sp1sum, \
             tc.tile_pool(name="sums", bufs=1, space="PSUM") as smpsum, \
             tc.tile_pool(name="avps", bufs=2, space="PSUM") as avpsum, \
             tc.tile_pool(name="yps", bufs=2, space="PSUM") as ypsum:

            wo_sb = wopool.tile([128, QH * D], BF16)
            nc.sync.dma_start(wo_sb[:], woP)

            def emit_wo(att_prev, b_prev, q0_prev):
                for tcx in range(QB // 128):
                    tg0 = b_prev * S + q0_prev + tcx * 128
                    for dp in range(D // (2 * NW)):
                        # two PSUM banks staged into one 1024-wide store
                        ysb = ypool.tile([128, 2 * NW], BF16)
                        for half in range(2):
                            dg = 2 * dp + half
                            yp = ypsum.tile([128, NW], F32)
                            for f in range(QH):
                                nc.tensor.matmul(
                                    yp[:],
                                    att_prev[f][:, tcx * 128:(tcx + 1) * 128],
                                    wo_sb[:, f * D + dg * NW:
                                          f * D + (dg + 1) * NW],
                                    start=(f == 0), stop=(f == QH - 1))
                            nc.vector.tensor_copy(
                                ysb[:, half * NW:(half + 1) * NW], yp[:])
                        nc.sync.dma_start(
                            y[tg0:tg0 + 128, 2 * dp * NW:(2 * dp + 2) * NW],
                            ysb[:])

            pending = None
            for b in range(B):
                for qb in range(S // QB):
                    q0 = qb * QB
                    nkt = (qb + 1) * (QB // 128)  # causal 128-wide kt chunks
                    att = [atpool.tile([128, QB], BF16, tag=f"att{h}",
                                       name=f"att{h}") for h in range(QH)]
                    # chunks are processed in alternating pair/single groups:
                    # a pair packs two score matmuls into one 2-bank PSUM
                    # tile and runs ONE exp over both, halving the scalar
                    # engine's per-instruction overhead so exp throughput
                    # stays ahead of the PE's S+AV+sums stream.
                    # only full-width chunks may pair: the second matmul of a
                    # pair must start exactly at the 512-element PSUM bank
                    # boundary (a matmul output cannot straddle banks)
                    grps = []
                    j = 0
                    pair = False     # lead with a single: first AV waits
                    while j < nkt:   # only a one-chunk exp
                        full2 = j + 1 < nkt and j + 1 < 4 * qb + 1
                        take = 2 if (pair and full2) else 1
                        grps.append(tuple(range(j, j + take)))
                        j += take
                        pair = not pair
                    for h in range(QH):
                        avp = avpsum.tile([128, QB], F32)
                        smp = smpsum.tile([1, QB], F32)
                        pts = {}

                        def emit_group(g):
                            pool = sp2sum if len(g) == 2 else sp1sum
                            stp = pool.tile([128, len(g) * QB], F32)
                            pt = ptpool.tile([128, 2 * QB], BF16,
                                             tag=f"pt{len(g)}",
                                             name=f"pt{len(g)}")
                            off = 0
                            for j in g:
                                r = j - 4 * qb   # >=0 on diagonal chunks
                                lo = max(r, 0) * 128
                                w = QB - lo
                                nc.tensor.matmul(
                                    stp[:, off:off + w],
                                    ktb[b][:, j * 128:(j + 1) * 128],
                                    qtb[h][b][:, q0 + lo:q0 + lo + w],
                                    start=True, stop=True)
                                if r >= 0:
                                    nc.vector.tensor_add(
                                        stp[:, off:off + 128],
                                        stp[:, off:off + 128], mtri[:])
                                pts[j] = (pt, off, lo, w)
                                off += w
                            nc.scalar.activation(pt[:, :off], stp[:, :off],
                                                 EXP, scale=SCALE)

                        gi = 0
                        while gi < len(grps) and gi < 2:
                            emit_group(grps[gi])
                            gi += 1
                        for gj, g in enumerate(grps):
                            for ktc in g:
                                pt, off, lo, w = pts.pop(ktc)
                                nc.tensor.matmul(
                                    avp[:, lo:QB],
                                    Vt[b][:, ktc * 128:(ktc + 1) * 128],
                                    pt[:, off:off + w],
                                    start=(ktc == 0), stop=(ktc == nkt - 1))
                                nc.tensor.matmul(
                                    smp[:, lo:QB], ones_t[:],
                                    pt[:, off:off + w],
                                    start=(ktc == 0), stop=(ktc == nkt - 1))
                            if gi < len(grps):
                                emit_group(grps[gi])
                                gi += 1
                        # normalize while evicting: att = avp * (1/sums)
                        s_sb = smpool.tile([1, QB], F32, tag="s_sb")
                        nc.vector.tensor_copy(s_sb[:], smp[:])
                        s_bc = smpool.tile([128, QB], F32, tag="s_bc")
                        nc.gpsimd.partition_broadcast(s_bc[:], s_sb[:])
                        r_bc = smpool.tile([128, QB], F32, tag="r_bc")
                        nc.vector.reciprocal_approx_fast(r_bc[:], s_bc[:])
                        nc.vector.tensor_mul(att[h][:], avp[:], r_bc[:])
                    # previous block's output projection queues behind this
                    # block's attention on the PE, hiding the normalize chain
                    if pending is not None:
                        emit_wo(*pending)
                    pending = (att, b, q0)
                    if epilogue is not None:
                        for xsb, xsw, out_ap in epilogue[0]:
                            rope_math(xsb, xsw, out_ap, epilogue[1])
                        epilogue = None
            if pending is not None:
                emit_wo(*pending)
    nc.compile()
    return nc


_program = None


def _get_program():
    global _program
    if _program is None:
        _program = _build_program()
    return _program


def kernel(**inputs) -> np.ndarray:
    x = np.asarray(inputs["x"], dtype=np.float32)
    wq = np.asarray(inputs["wq"], dtype=np.float32)
    wk = np.asarray(inputs["wk"], dtype=np.float32)
    wv = np.asarray(inputs["wv"], dtype=np.float32)
    wo = np.asarray(inputs["wo"], dtype=np.float32)
    cos = np.asarray(inputs["freqs_cos"], dtype=np.float32)
    sin = np.asarray(inputs["freqs_sin"], dtype=np.float32)
    start_pos = int(np.asarray(inputs.get("start_pos", 0)))
    assert start_pos == 0, "kernel specialized for start_pos == 0"

    # Even/odd RoPE pair split within each head's 128 features.
    perm = np.concatenate([np.arange(0, HD, 2), np.arange(1, HD, 2)])

    xT = np.ascontiguousarray(x.reshape(T, D).T.astype(NPBF16))
    cosT = cos.T                                   # [64, S]
    sinT = sin.T
    ropc = np.ascontiguousarray(np.concatenate([cosT, cosT], axis=0))
    rops = np.ascontiguousarray(np.concatenate([-sinT, sinT], axis=0))
    rr = np.arange(128)
    maskin = np.where(rr[None, :] >= rr[:, None], 0.0, NEG).astype(NPBF16)

    def pack(wT, width):
        # [D, width] -> [128, DKD * width], k-chunk-major
        return np.ascontiguousarray(
            wT.reshape(DKD, 128, width).transpose(1, 0, 2)
            .reshape(128, DKD * width).astype(NPBF16))

    in_maps = []
    for c in range(N_CORES):
        wq_c = wq[c * FL:(c + 1) * FL].reshape(QH, HD, D)[:, perm, :].reshape(FL, D)
        wk_c = wk[c * HD:(c + 1) * HD][perm, :]
        wv_c = wv[c * HD:(c + 1) * HD]
        wo_cT = wo[:, c * FL:(c + 1) * FL].T       # [FL, D]
        in_maps.append({
            "xT": xT,
            "wqP": pack(wq_c.T, FL),
            "wkP": pack(wk_c.T, HD),
            "wvP": pack(wv_c.T, HD),
            "woP": np.ascontiguousarray(
                wo_cT.reshape(QH, 128, D).transpose(1, 0, 2)
                .reshape(128, QH * D).astype(NPBF16)),
            "ropc": ropc,
            "rops": rops,
            "onesin": np.ones((128, 1), dtype=NPBF16),
            "maskin": maskin,
        })

    nc = _get_program()
    trace = bool(int(os.environ.get("GQA_TRACE", "0")))
    kwargs = {}
    if trace:
        tmpdir = os.environ.get("GQA_TRACE_DIR") or None
        kwargs = dict(trace=True, tmpdir=tmpdir, trace_cores=[0])
    res = run_bass_kernel_spmd(nc, in_maps, list(range(N_CORES)), **kwargs)
    kernel.last_results = res

    acc = np.zeros((T, D), dtype=np.float32)
    for c in range(N_CORES):
        acc += res.results[c]["y"].astype(np.float32)
    return acc.reshape(B, S, D)


# revision 21
# speedup vs baseline: 1.0730x; 1.0730x over previous
"""GQA attention layer (B=2, S=2048, D=4096, 32 Q heads / 8 KV heads, RoPE,
causal) on 8 Trainium2 NeuronCores, tensor-parallel over heads.

Each core owns 4 Q heads + 1 KV head: it computes its Q/K/V projections,
RoPE, causal attention, and a partial output projection (rank-512 slice of
the wo contraction).  The host sums the 8 partial outputs.

v2 (all-bf16, SBUF-resident):
- every SBUF operand in bf16 (fp32 PSUM accumulation): halves DMA traffic,
  enables fast weight loads, and lowers PE power draw (the fp32r baseline
  ran power-throttled at K=13/16 for half the kernel).
- q/k/v and the token-major V live entirely in SBUF; no DRAM round trip
  between the projection and attention phases.
- causal trimming: diagonal score/AV/sum matmuls shrink their moving
  width to 512/384/256/128 instead of running fully masked.
- softmax normalization folded into the PSUM eviction of the attention
  output (broadcast + reciprocal + one fused multiply).
- PE warm-up matmuls gated only on a tiny mask DMA so the HAM clock gate
  opens before the real work arrives; weights stream per k-chunk so the
  first projection matmul does not wait for whole-tensor DMAs.
"""

import os
import sys
import types
from contextlib import ExitStack

import numpy as np
import ml_dtypes

import concourse.bass as bass
import concourse.tile as tile
from concourse import bacc
from concourse import mybir
from concourse import bass_utils
from concourse.bass_utils import run_bass_kernel_spmd

# ---------------------------------------------------------------------------
# Optional NTFF profiling support under axon. The trimmed image's `antenv`
# lacks `axon_hooks`, so run_bass_kernel_spmd(trace=True) would silently skip
# tracing; register the hook ourselves. Harmless when unavailable.
try:
    import antenv  # noqa: F401
    from trn_agent_boot.trn_boot import _ntff_profile_via_ctypes

    if "antenv.axon_hooks" not in sys.modules:
        _hooks_mod = types.ModuleType("antenv.axon_hooks")
        _hook = _ntff_profile_via_ctypes("/opt/axon/libaxon_pjrt.so")
        _hooks_mod.get_axon_ntff_profile_hook = lambda: _hook
        _hooks_mod.set_axon_ntff_profile_hook = lambda h: None
        sys.modules["antenv.axon_hooks"] = _hooks_mod
    bass_utils.upload_artifacts = lambda tmpdir: "local://skipped"
except Exception:
    pass

F32 = mybir.dt.float32
BF16 = mybir.dt.bfloat16
EXP = mybir.ActivationFunctionType.Exp
NPBF16 = ml_dtypes.bfloat16

B, S, D = 2, 2048, 4096
NH, NKV, HD = 32, 8, 128
T = B * S                       # 4096 tokens total
N_CORES = 8
QH = NH // N_CORES              # 4 local q heads
FL = QH * HD                    # 512 local q features
SCALE = 1.0 / float(np.sqrt(HD))
NEG = -1.0e30

NW = 512                        # token-group width in the QKV projection
QB = 512                        # q-block width in attention
DKD = D // 128                  # 32 contraction chunks for projections


def _build_program():
    nc = bacc.Bacc("TRN2", target_bir_lowering=False, debug=False,
                   num_devices=N_CORES)

    xT = nc.dram_tensor("xT", [D, T], BF16, kind="ExternalInput").ap()
    # weights pre-packed on host, k-chunk-major: [128, n_chunks * width]
    wqP = nc.dram_tensor("wqP", [128, DKD * FL], BF16, kind="ExternalInput").ap()
    wkP = nc.dram_tensor("wkP", [128, DKD * HD], BF16, kind="ExternalInput").ap()
    wvP = nc.dram_tensor("wvP", [128, DKD * HD], BF16, kind="ExternalInput").ap()
    woP = nc.dram_tensor("woP", [128, QH * D], BF16, kind="ExternalInput").ap()
    # RoPE constants, pre-assembled for the rotate-half formulation on the
    # even/odd-split feature layout: ropc = [cos; cos], rops = [-sin; sin].
    ropc = nc.dram_tensor("ropc", [HD, S], F32, kind="ExternalInput").ap()
    rops = nc.dram_tensor("rops", [HD, S], F32, kind="ExternalInput").ap()
    onesin = nc.dram_tensor("onesin", [128, 128], BF16, kind="ExternalInput").ap()
    # triangular mask for a diagonal 128x128 block: m[r, c] = 0 if c >= r
    maskin = nc.dram_tensor("maskin", [128, 128], BF16, kind="ExternalInput").ap()
    y = nc.dram_tensor("y", [T, D], BF16, kind="ExternalOutput").ap()

    with tile.TileContext(nc) as tc, ExitStack() as ctx:
        const = ctx.enter_context(tc.tile_pool(name="const", bufs=1))
        mtri = const.tile([128, 128], BF16)
        nc.sync.dma_start(mtri[:], maskin)
        ones_t = const.tile([128, 128], BF16)
        nc.sync.dma_start(ones_t[:], onesin)

        # PE warm-up: ~10us of dummy matmuls gated only on the tiny mask DMA,
        # so the HAM clock gate reaches K=8/8 before the first real matmul.
        with tc.tile_pool(name="warmps", bufs=1, space="PSUM") as wps:
            wtile = wps.tile([128, 128], F32)
            for _ in range(96):
                nc.tensor.matmul(wtile[:], mtri[:], mtri[:],
                                 start=True, stop=True)

        # Resident weights (DMAs emitted inside the phase-1 loop, interleaved
        # with the x-chunk loads so the sync queue serves them in need order).
        wpool = ctx.enter_context(tc.tile_pool(name="wqkv", bufs=1))
        wq_sb = wpool.tile([128, DKD * FL], BF16, tag="wq")
        wk_sb = wpool.tile([128, DKD * HD], BF16, tag="wk")
        wv_sb = wpool.tile([128, DKD * HD], BF16, tag="wv")

        # rope tables on the scalar hwdge queue, off the x/weight path
        rcpool = ctx.enter_context(tc.tile_pool(name="ropec", bufs=1))
        cos_s = rcpool.tile([HD, S], F32)
        nc.scalar.dma_start(cos_s[:], ropc)
        sin_s = rcpool.tile([HD, S], F32)
        nc.scalar.dma_start(sin_s[:], rops)

        # Resident activations (bf16, feature-major), split per batch so the
        # batch-0 attention does not depend on batch-1 projection writes.
        res = ctx.enter_context(tc.tile_pool(name="resident", bufs=1))
        qtb = [[res.tile([128, S], BF16, tag=f"qtb{h}_{b}", name=f"qtb{h}_{b}")
                for b in range(B)] for h in range(QH)]
        ktb = [res.tile([128, S], BF16, tag=f"ktb{b}", name=f"ktb{b}")
               for b in range(B)]
        Vt = [res.tile([128, S], BF16, tag=f"Vt{b}", name=f"Vt{b}")
              for b in range(B)]                  # token-major V blocks

        # ------------------------------------------------------------------
        # Phase 1: QKV projections + RoPE -> resident SBUF tiles
        # ------------------------------------------------------------------
        # Phase 1 runs in 1024-token group pairs: x chunks load once per pair
        # as [128, 1024] transfers (2 KiB rows) and stay resident for both
        # groups. Rope math for a group is deferred one group so its
        # swap-DMA-dependent vector ops never delay the PSUM-freeing copies.
        rtmp = ctx.enter_context(tc.tile_pool(name="ropetmp", bufs=2))
        with tc.tile_pool(name="xin", bufs=1) as xpool, \
             tc.tile_pool(name="qkvps", bufs=1, space="PSUM") as qkvps:

            def rope_math(xsb, xsw, out_ap, pos0):
                """out = RoPE(xsb) on the even/odd-split feature layout:
                out = x * [c;c] + swap(x) * [-s;s]; xsw = swap(x) (already
                staged by DMA)."""
                c = cos_s[:, pos0:pos0 + NW]
                s = sin_s[:, pos0:pos0 + NW]
                t1 = rtmp.tile([128, NW], BF16, tag="t1")
                nc.vector.tensor_mul(t1[:], xsw[:], s)
                nc.vector.tensor_mul(out_ap, xsb[:], c)
                nc.vector.tensor_add(out_ap, out_ap, t1[:])

            deferred = None
            pair_xts = None
            for n in range(T // NW):
                b = n // (S // NW)
                pos0 = (n * NW) % S
                g0 = n * NW                      # global column
                pcol = (n % 2) * NW              # column within the pair tile
                qps = [qkvps.tile([128, NW], F32, tag=f"qps{m}", name=f"qps{m}")
                       for m in range(QH)]
                kps = qkvps.tile([128, NW], F32, tag="kps")
                vps = qkvps.tile([128, NW], F32, tag="vps")
                if n % 2 == 0:
                    pair_xts = []
                for k in range(DKD):
                    if n == 0:
                        # weight chunks land just ahead of their x chunk
                        if k % 8 == 0:
                            q8 = 8 * FL
                            nc.sync.dma_start(
                                wq_sb[:, (k // 8) * q8:(k // 8 + 1) * q8],
                                wqP[:, (k // 8) * q8:(k // 8 + 1) * q8])
                        if k == 0:
                            nc.sync.dma_start(wk_sb[:], wkP)
                            nc.sync.dma_start(wv_sb[:], wvP)
                    if n % 2 == 0:
                        xt = xpool.tile([128, 2 * NW], BF16, tag=f"xt{k}",
                                        name=f"xt{k}")
                        nc.sync.dma_start(
                            xt[:], xT[k * 128:(k + 1) * 128, g0:g0 + 2 * NW])
                        pair_xts.append(xt)
                    else:
                        xt = pair_xts[k]
                    st = (k == 0)
                    sp = (k == DKD - 1)
                    for m in range(QH):
                        nc.tensor.matmul(
                            qps[m][:],
                            wq_sb[:, k * FL + m * 128:k * FL + (m + 1) * 128],
                            xt[:, pcol:pcol + NW], start=st, stop=sp)
                    nc.tensor.matmul(
                        kps[:], wk_sb[:, k * HD:(k + 1) * HD],
                        xt[:, pcol:pcol + NW], start=st, stop=sp)
                    nc.tensor.matmul(
                        vps[:], wv_sb[:, k * HD:(k + 1) * HD],
                        xt[:, pcol:pcol + NW], start=st, stop=sp)
                # PSUM-freeing copies first (split across ACT and DVE) so the
                # next group's matmuls are only gated on these, then the swap
                # DMAs; the swap-dependent math runs one group later.
                ev = []
                for m in range(QH):
                    xsb = rtmp.tile([128, NW], BF16, tag=f"xsb{m}",
                                    name=f"xsb{m}")
                    if m % 2 == 1:
                        nc.vector.tensor_copy(xsb[:], qps[m][:])
                    else:
                        nc.scalar.copy(xsb[:], qps[m][:])
                    xsw = rtmp.tile([128, NW], BF16, tag=f"xsw{m}",
                                    name=f"xsw{m}")
                    # swaps ride the gpsimd software DGE: queue-full waits
                    # there cannot block the scalar/vector compute FIFOs
                    nc.gpsimd.dma_start(xsw[0:64, :], xsb[64:128, :])
                    nc.gpsimd.dma_start(xsw[64:128, :], xsb[0:64, :])
                    ev.append((xsb, xsw, qtb[m][b][:, pos0:pos0 + NW]))
                xk = rtmp.tile([128, NW], BF16, tag="xsbk")
                nc.vector.tensor_copy(xk[:], kps[:])
                xkw = rtmp.tile([128, NW], BF16, tag="xswk")
                nc.gpsimd.dma_start(xkw[0:64, :], xk[64:128, :])
                nc.gpsimd.dma_start(xkw[64:128, :], xk[0:64, :])
                ev.append((xk, xkw, ktb[b][:, pos0:pos0 + NW]))
                xv = rtmp.tile([128, NW], BF16, tag="xsbv")
                nc.scalar.copy(xv[:], vps[:])
                for j in range(NW // 128):
                    nc.sync.dma_start_transpose(
                        Vt[b][:, pos0 + j * 128:pos0 + (j + 1) * 128],
                        xv[:, j * 128:(j + 1) * 128])
                if deferred is not None:
                    for xsb, xsw, out_ap in deferred[0]:
                        rope_math(xsb, xsw, out_ap, deferred[1])
                deferred = (ev, pos0)
            # the last group's rope math (batch-1 tail) is emitted in the
            # middle of phase 2's first block, after its congested swap DMAs
            # have landed, so it cannot delay the first attention vector ops
            epilogue = deferred

        # ------------------------------------------------------------------
        # Phase 2: attention + output projection (wo deferred one block)
        # ------------------------------------------------------------------
        with tc.tile_pool(name="wo", bufs=1) as wopool, \
             tc.tile_pool(name="ptiles", bufs=3) as ptpool, \
             tc.tile_pool(name="attn", bufs=2) as atpool, \
             tc.tile_pool(name="smax", bufs=2) as smpool, \
             tc.tile_pool(name="ystage", bufs=3) as ypool, \
             tc.tile_pool(name="sps2", bufs=1, space="PSUM") as sp2sum, \
             tc.tile_pool(name="sps1", bufs=1, space="PSUM") as sp1sum, \
             tc.tile_pool(name="sums", bufs=1, space="PSUM") as smpsum, \
             tc.tile_pool(name="avps", bufs=2, space="PSUM") as avpsum, \
             tc.tile_pool(name="yps", bufs=2, space="PSUM") as ypsum:

            wo_sb = wopool.tile([128, QH * D], BF16)
            nc.sync.dma_start(wo_sb[:], woP)

            def emit_wo(att_prev, b_prev, q0_prev):
                for tcx in range(QB // 128):
                    tg0 = b_prev * S + q0_prev + tcx * 128
                    for dp in range(D // (2 * NW)):
                        # two PSUM banks staged into one 1024-wide store
                        ysb = ypool.tile([128, 2 * NW], BF16)
                        for half in range(2):
                            dg = 2 * dp + half
                            yp = ypsum.tile([128, NW], F32)
                            for f in range(QH):
                                nc.tensor.matmul(
                                    yp[:],
                                    att_prev[f][:, tcx * 128:(tcx + 1) * 128],
                                    wo_sb[:, f * D + dg * NW:
                                          f * D + (dg + 1) * NW],
                                    start=(f == 0), stop=(f == QH - 1))
                            nc.vector.tensor_copy(
                                ysb[:, half * NW:(half + 1) * NW], yp[:])
                        nc.sync.dma_start(
                            y[tg0:tg0 + 128, 2 * dp * NW:(2 * dp + 2) * NW],
                            ysb[:])

            pending = None
            for b in range(B):
                for qb in range(S // QB):
                    q0 = qb * QB
                    nkt = (qb + 1) * (QB // 128)  # causal 128-wide kt chunks
                    att = [atpool.tile([128, QB], BF16, tag=f"att{h}",
                                       name=f"att{h}") for h in range(QH)]
                    # chunks are processed in alternating pair/single groups:
                    # a pair packs two score matmuls into one 2-bank PSUM
                    # tile and runs ONE exp over both, halving the scalar
                    # engine's per-instruction overhead so exp throughput
                    # stays ahead of the PE's S+AV+sums stream.
                    # only full-width chunks may pair: the second matmul of a
                    # pair must start exactly at the 512-element PSUM bank
                    # boundary (a matmul output cannot straddle banks)
                    grps = []
                    j = 0
                    pair = False     # lead with a single: first AV waits
                    while j < nkt:   # only a one-chunk exp
                        full2 = j + 1 < nkt and j + 1 < 4 * qb + 1
                        take = 2 if (pair and full2) else 1
                        grps.append(tuple(range(j, j + take)))
                        j += take
                        pair = not pair
                    for h in range(QH):
                        avp = avpsum.tile([128, QB], F32)
                        smp = smpsum.tile([128, QB], F32)
                        pts = {}

                        def emit_group(g):
                            pool = sp2sum if len(g) == 2 else sp1sum
                            stp = pool.tile([128, len(g) * QB], F32)
                            pt = ptpool.tile([128, 2 * QB], BF16,
                                             tag=f"pt{len(g)}",
                                             name=f"pt{len(g)}")
                            off = 0
                            for j in g:
                                r = j - 4 * qb   # >=0 on diagonal chunks
                                lo = max(r, 0) * 128
                                w = QB - lo
                                nc.tensor.matmul(
                                    stp[:, off:off + w],
                                    ktb[b][:, j * 128:(j + 1) * 128],
                                    qtb[h][b][:, q0 + lo:q0 + lo + w],
                                    start=True, stop=True)
                                if r >= 0:
                                    nc.vector.tensor_add(
                                        stp[:, off:off + 128],
                                        stp[:, off:off + 128], mtri[:])
                                pts[j] = (pt, off, lo, w)
                                off += w
                            nc.scalar.activation(pt[:, :off], stp[:, :off],
                                                 EXP, scale=SCALE)

                        gi = 0
                        while gi < len(grps) and gi < 2:
                            emit_group(grps[gi])
                            gi += 1
                        for gj, g in enumerate(grps):
                            for ktc in g:
                                pt, off, lo, w = pts.pop(ktc)
                                nc.tensor.matmul(
                                    avp[:, lo:QB],
                                    Vt[b][:, ktc * 128:(ktc + 1) * 128],
                                    pt[:, off:off + w],
                                    start=(ktc == 0), stop=(ktc == nkt - 1))
                                nc.tensor.matmul(
                                    smp[:, lo:QB], ones_t[:],
                                    pt[:, off:off + w],
                                    start=(ktc == 0), stop=(ktc == nkt - 1))
                            if gi < len(grps):
                                emit_group(grps[gi])
                                gi += 1
                        # normalize while evicting: att = avp * (1/sums);
                        # the all-ones stationary already wrote the sums to
                        # every partition, so no cross-partition broadcast
                        s_bc = smpool.tile([128, QB], F32, tag="s_bc")
                        nc.vector.tensor_copy(s_bc[:], smp[:])
                        r_bc = smpool.tile([128, QB], F32, tag="r_bc")
                        nc.vector.reciprocal_approx_fast(r_bc[:], s_bc[:])
                        nc.vector.tensor_mul(att[h][:], avp[:], r_bc[:])
                    # previous block's output projection queues behind this
                    # block's attention on the PE, hiding the normalize chain
                    if pending is not None:
                        emit_wo(*pending)
                    pending = (att, b, q0)
                    if epilogue is not None:
                        for xsb, xsw, out_ap in epilogue[0]:
                            rope_math(xsb, xsw, out_ap, epilogue[1])
                        epilogue = None
            if pending is not None:
                emit_wo(*pending)
    nc.compile()
    return nc


_program = None


def _get_program():
    global _program
    if _program is None:
        _program = _build_program()
    return _program


def kernel(**inputs) -> np.ndarray:
    x = np.asarray(inputs["x"], dtype=np.float32)
    wq = np.asarray(inputs["wq"], dtype=np.float32)
    wk = np.asarray(inputs["wk"], dtype=np.float32)
    wv = np.asarray(inputs["wv"], dtype=np.float32)
    wo = np.asarray(inputs["wo"], dtype=np.float32)
    cos = np.asarray(inputs["freqs_cos"], dtype=np.float32)
    sin = np.asarray(inputs["freqs_sin"], dtype=np.float32)
    start_pos = int(np.asarray(inputs.get("start_pos", 0)))
    assert start_pos == 0, "kernel specialized for start_pos == 0"

    # Even/odd RoPE pair split within each head's 128 features.
    perm = np.concatenate([np.arange(0, HD, 2), np.arange(1, HD, 2)])

    xT = np.ascontiguousarray(x.reshape(T, D).T.astype(NPBF16))
    cosT = cos.T                                   # [64, S]
    sinT = sin.T
    ropc = np.ascontiguousarray(np.concatenate([cosT, cosT], axis=0))
    rops = np.ascontiguousarray(np.concatenate([-sinT, sinT], axis=0))
    rr = np.arange(128)
    maskin = np.where(rr[None, :] >= rr[:, None], 0.0, NEG).astype(NPBF16)

    def pack(wT, width):
        # [D, width] -> [128, DKD * width], k-chunk-major
        return np.ascontiguousarray(
            wT.reshape(DKD, 128, width).transpose(1, 0, 2)
            .reshape(128, DKD * width).astype(NPBF16))

    in_maps = []
    for c in range(N_CORES):
        wq_c = wq[c * FL:(c + 1) * FL].reshape(QH, HD, D)[:, perm, :].reshape(FL, D)
        wk_c = wk[c * HD:(c + 1) * HD][perm, :]
        wv_c = wv[c * HD:(c + 1) * HD]
        wo_cT = wo[:, c * FL:(c + 1) * FL].T       # [FL, D]
        in_maps.append({
            "xT": xT,
            "wqP": pack(wq_c.T, FL),
            "wkP": pack(wk_c.T, HD),
            "wvP": pack(wv_c.T, HD),
            "woP": np.ascontiguousarray(
                wo_cT.reshape(QH, 128, D).transpose(1, 0, 2)
                .reshape(128, QH * D).astype(NPBF16)),
            "ropc": ropc,
            "rops": rops,
            "onesin": np.ones((128, 128), dtype=NPBF16),
            "maskin": maskin,
        })

    nc = _get_program()
    trace = bool(int(os.environ.get("GQA_TRACE", "0")))
    kwargs = {}
    if trace:
        tmpdir = os.environ.get("GQA_TRACE_DIR") or None
        kwargs = dict(trace=True, tmpdir=tmpdir, trace_cores=[0])
    res = run_bass_kernel_spmd(nc, in_maps, list(range(N_CORES)), **kwargs)
    kernel.last_results = res

    acc = np.zeros((T, D), dtype=np.float32)
    for c in range(N_CORES):
        acc += res.results[c]["y"].astype(np.float32)
    return acc.reshape(B, S, D)
